# revision 52
# baseline (speedup 1.0000x reference)
"""Causal multi-head attention (B=4, N=2048, D=768, H=12) on 8 TRN2 cores.

Sharding: batch (4) x head-split (2). Core c = (b, hg) handles batch b
and heads 6*hg .. 6*hg+5 over the FULL sequence: QKV projections take
only this half's weight columns, attention runs 6 heads, and the out
projection uses only this half's weight rows, producing a partial
[N, D] that the host sums across the pair of cores sharing a batch.

All matmul operands are bf16 (fp32 PSUM accumulation). Per core:
  qT/kT = Wqk^T x^T  ([2*64, N] head-pair tiles)    v = x Wv (+ones col)
  per (head, query-half, key-tile jt), software-pipelined so the PE
  never waits on the Act engine (scores for jt+1 issue before AV jt):
    s = kT_jt^T qT;  et = exp(s/8) bf16 (* 0/1 causal mask on diag
    tiles, DVE);  oT[65, :] += v_aug_jt^T et  (row 64 = denominators)
  aT = oT[0:64] (pair-packed via DMA repartition for odd heads)
  rec broadcast via PE selector-matmul into PSUM (deferred into the
  next head's loop);  aTb = aT * rec  bf16;   o_partial = aTb^T Wo
"""

import numpy as np

B, N, D, H = 4, 2048, 768, 12
DH = D // H          # 64
HH = H // 2          # 6 local heads per core
NPAIR = HH // 2      # 3 head pairs
KC = D // 128        # 6 contraction chunks
VW = HH * (DH + 1)   # 390 (v_aug row width per seq tile)
NEG = -30000.0

_CACHE = {}


def _build_nc(unused=None):
    import concourse.bacc as bacc
    import concourse.bass as bass
    import concourse.mybir as mybir
    import concourse.tile as tile
    from contextlib import ExitStack

    dt = mybir.dt
    f32 = dt.float32
    bf16 = dt.bfloat16
    fp8 = dt.float8e4
    Exp = mybir.ActivationFunctionType.Exp

    nc = bacc.Bacc(None)
    xt = nc.declare_dram_parameter("xt", [D, N], bf16, isOutput=False)
    wqk = nc.declare_dram_parameter("wqk", [D, 2 * HH * DH], bf16,
                                    isOutput=False)
    wv = nc.declare_dram_parameter("wv", [D, HH * DH], bf16, isOutput=False)
    wo = nc.declare_dram_parameter("wo", [HH * DH, D], bf16, isOutput=False)
    cst = nc.declare_dram_parameter("cst", [2, 128, 128], bf16, isOutput=False)
    o = nc.declare_dram_parameter("o", [N, D], bf16, isOutput=True)

    with tile.TileContext(nc) as tc:
        with ExitStack() as es:
            persist = es.enter_context(tc.tile_pool(name="persist", bufs=1))
            qT = [persist.tile([128, N], bf16, tag=f"qT{m}", name=f"qT{m}")
                  for m in range(NPAIR)]
            kT = [persist.tile([128, N], bf16, tag=f"kT{m}", name=f"kT{m}")
                  for m in range(NPAIR)]
            vp_all = persist.tile([128, 16 * VW], bf16, tag="vp", name="vp")
            vp = [vp_all[:, j * VW:(j + 1) * VW] for j in range(16)]
            aT = [persist.tile([128, N], f32, tag=f"aT{m}", name=f"aT{m}")
                  for m in range(NPAIR)]
            aTb = [persist.tile([128, N], bf16, tag=f"aTb{m}", name=f"aTb{m}")
                   for m in range(NPAIR)]
            # den/rec rows live at partitions 0 (even head) and 64 (odd
            # head) so the PE rec-broadcast lhsT/rhs base partitions align.
            den = [persist.tile([65, N], f32, tag=f"den{m}", name=f"den{m}")
                   for m in range(NPAIR)]
            rec = [persist.tile([65, N], f32, tag=f"rec{m}", name=f"rec{m}")
                   for m in range(NPAIR)]
            recb = persist.tile([128, N], bf16, tag="recb", name="recb")
            for m in range(NPAIR):
                nc.vector.memset(den[m][:], 1.0)
            nc.vector.memset(recb[:], 1.0)
            msk = persist.tile([128, 256], bf16, tag="msk", name="msk")
            tri01 = msk[:, 0:128]
            sel = msk[:, 128:256]
            wos = persist.tile([128, NPAIR * D], bf16, tag="wos", name="wos")

            # ---------------- projections ----------------
            with tc.tile_pool(name="xw", bufs=1) as xwp, \
                 tc.tile_pool(name="on", bufs=1) as onp, \
                 tc.tile_pool(name="pp", bufs=1, space="PSUM") as pp, \
                 tc.tile_pool(name="ppv", bufs=2, space="PSUM") as ppv:
                xts = xwp.tile([128, KC * N], bf16, tag="xts", name="xts")
                wqks = xwp.tile([128, KC * 768], bf16, tag="wqks",
                                name="wqks")
                wvs = xwp.tile([128, KC * 384], bf16, tag="wvs", name="wvs")
                for k in range(KC):
                    nc.scalar.dma_start(out=wqks[:, k * 768:(k + 1) * 768],
                                        in_=wqk[k * 128:(k + 1) * 128, :])
                    nc.sync.dma_start(
                        out=xts[:, k * N:k * N + 512],
                        in_=xt[k * 128:(k + 1) * 128, 0:512])

                # q/k projection, k-major: 6 output tiles accumulate in
                # PSUM while the k-chunk DMAs stream in behind them
                for qtr in range(4):
                    c0 = qtr * 512
                    if qtr > 0:
                        for k in range(KC):
                            nc.sync.dma_start(
                                out=xts[:, k * N + c0:k * N + c0 + 512],
                                in_=xt[k * 128:(k + 1) * 128, c0:c0 + 512])
                    pss = [pp.tile([128, 512], f32, tag=f"psq{m}",
                                   name=f"psq{m}") for m in range(2 * NPAIR)]
                    for k in range(KC):
                        for m in range(2 * NPAIR):
                            nc.tensor.matmul(
                                out=pss[m][:],
                                lhsT=wqks[:, k * 768 + m * 128:
                                          k * 768 + (m + 1) * 128],
                                rhs=xts[:, k * N + c0:k * N + c0 + 512],
                                start=(k == 0), stop=(k == KC - 1))
                    for m in range(2 * NPAIR):
                        dst = qT[m] if m < NPAIR else kT[m - NPAIR]
                        nc.vector.tensor_copy(dst[:, c0:c0 + 512],
                                              pss[m][:])
                for k in range(KC):
                    nc.gpsimd.dma_start(out=wvs[:, k * 384:(k + 1) * 384],
                                        in_=wv[k * 128:(k + 1) * 128, :])
                for i2 in range(2):
                    nc.sync.dma_start(out=msk[:, i2 * 128:(i2 + 1) * 128],
                                      in_=cst[i2])
                for f in range(NPAIR):
                    nc.gpsimd.dma_start(out=wos[:, f * D:(f + 1) * D],
                                        in_=wo[f * 128:(f + 1) * 128, :])

                # v projection: out [seq 128, 6*64] per seq tile
                for st in range(16):
                    ps = ppv.tile([128, 384], f32, tag="psv", name="psv")
                    for k in range(KC):
                        nc.tensor.matmul(
                            out=ps[:],
                            lhsT=xts[:, k * N + st * 128:
                                     k * N + (st + 1) * 128],
                            rhs=wvs[:, k * 384:(k + 1) * 384],
                            start=(k == 0), stop=(k == KC - 1))
                    vv = vp[st].rearrange("p (h c) -> p h c", c=DH + 1)
                    nc.vector.tensor_copy(
                        vv[:, :, 0:DH],
                        ps[:].rearrange("p (h c) -> p h c", c=DH))
                ones96 = onp.tile([128, 16 * HH], bf16, tag="on",
                                  name="ones96")
                nc.vector.memset(ones96[:], 1.0)
                vview = vp_all.rearrange("p (j c) -> p j c", c=DH + 1)
                nc.vector.tensor_copy(
                    vview[:, :, DH:DH + 1],
                    ones96[:].rearrange("p (a b) -> p a b", b=1))

            # ---------------- attention ----------------
            import concourse.bass as bass  # noqa: F401

            def r32(ap):
                return ap.bitcast(dt.float32r)

            with tc.tile_pool(name="et", bufs=5) as etp, \
                 tc.tile_pool(name="dtm", bufs=2) as dtp, \
                 tc.tile_pool(name="omp", bufs=2) as omp, \
                 tc.tile_pool(name="ps4", bufs=2, space="PSUM") as ps4, \
                 tc.tile_pool(name="po4", bufs=2, space="PSUM") as po4:
                # normalize PE-work (rec broadcast + multiply) is deferred
                # into a later jt loop so the PE never waits on the
                # DVE/DMA den->rec chain
                pnorm = []

                def flush_norm(pool):
                    while pnorm:
                        m_, q0_ = pnorm.pop(0)
                        rbps = pool.tile([128, 1024], f32, tag="ps",
                                         name="ps")
                        for (a, b) in ((0, 512), (512, 1024)):
                            nc.tensor.matmul(
                                out=rbps[:, a:b], lhsT=sel,
                                rhs=recb[:, q0_ + a:q0_ + b],
                                start=True, stop=True,
                                skip_group_check=True)
                        nc.vector.tensor_mul(
                            aTb[m_][:, q0_:q0_ + 1024],
                            aT[m_][:, q0_:q0_ + 1024], rbps[:])

                for m in range(NPAIR):
                    otmp = omp.tile([64, N], f32, tag="otmp", name="otmp")
                    for hh in range(2):
                        h = 2 * m + hh
                        hs = slice(hh * 64, hh * 64 + 64)
                        for qc in range(2):
                            q0 = qc * 1024
                            oT = po4.tile([65, 1024], f32, tag="oT",
                                          name="oT")
                            njt = 8 * qc + 8

                            def do_av(jt, et, regs):
                                for (a, b) in regs:
                                    nc.tensor.matmul(
                                        out=oT[:, a:b],
                                        lhsT=vp[jt][:, h * (DH + 1):
                                                    (h + 1) * (DH + 1)],
                                        rhs=et[:, a:b],
                                        start=(jt == 0),
                                        stop=(jt == (8 * qc + 3 if b <= 512
                                                     else njt - 1)),
                                        skip_group_check=True)

                            # jt-pair bursting: S,S then the previous
                            # pair's AV,AV — fewer stationary switches and
                            # semaphore-gated PE instructions per step
                            pend = []
                            for jt in range(njt):
                                ql0 = max(0, 128 * jt - q0)
                                diag = jt >= 8 * qc
                                regs = []
                                if ql0 < 512:
                                    regs.append((ql0, 512))
                                regs.append((max(ql0, 512), 1024))
                                if jt == (7 if qc == 0 else 6):
                                    flush_norm(ps4)
                                ps = ps4.tile([128, 1024], f32, tag="ps",
                                              name="ps")
                                for (a, b) in regs:
                                    nc.tensor.matmul(
                                        out=ps[:, a:b],
                                        lhsT=kT[m][hs, jt * 128:
                                                   (jt + 1) * 128],
                                        rhs=qT[m][hs, q0 + a:q0 + b],
                                        start=True, stop=True,
                                        skip_group_check=True)
                                et = etp.tile([128, 1024], bf16, tag="et",
                                              name="et")
                                nc.scalar.activation(
                                    out=et[:, ql0:1024], in_=ps[:, ql0:1024],
                                    func=Exp, scale=0.125)
                                if diag:
                                    nc.vector.tensor_mul(
                                        et[:, ql0:ql0 + 128],
                                        et[:, ql0:ql0 + 128], tri01)
                                pend.append((jt, et, regs))
                                if jt % 2 == 1 and len(pend) == 4:
                                    do_av(*pend.pop(0))
                                    do_av(*pend.pop(0))
                            for p_ in pend:
                                do_av(*p_)
                            # drain oT: rows 0..63 -> aT / otmp, row 64 -> den
                            if hh == 0:
                                nc.vector.tensor_copy(
                                    aT[m][0:64, q0:q0 + 1024], oT[0:64, :])
                            else:
                                nc.vector.tensor_copy(
                                    otmp[0:64, q0:q0 + 1024], oT[0:64, :])
                            dtm = dtp.tile([65, 1024], f32, tag="dtm",
                                           name="dtm")
                            nc.vector.tensor_copy(dtm[64:65, :], oT[64:65, :])
                            dr = hh * 64  # den row: partition 0 / 64
                            nc.sync.dma_start(
                                out=den[m][dr:dr + 1, q0:q0 + 1024],
                                in_=dtm[64:65, :])
                            if hh == 1:
                                # pair half complete: repartition odd head,
                                # then normalize this query-half
                                nc.sync.dma_start(
                                    out=aT[m][64:128, q0:q0 + 1024],
                                    in_=otmp[:, q0:q0 + 1024])
                                nc.vector.reciprocal_approx_fast(
                                    out=rec[m][:, q0:q0 + 1024],
                                    in_=den[m][:, q0:q0 + 1024])
                                nc.vector.tensor_copy(
                                    recb[0:65, q0:q0 + 1024],
                                    rec[m][:, q0:q0 + 1024])
                                pnorm.append((m, q0))

                # ------------- output projection -------------
                # runs inside the attention pools (ps/et tiles reused)
                # so no pool-close barrier stalls the PE at the seam
                for st in range(16):
                    if st == 1:
                        # last pair's deferred normalize: its PE ops land
                        # here so st=0's matmuls cover the DVE chain; only
                        # st>=8 reads the columns this mul produces
                        flush_norm(ps4)
                    ps = ps4.tile([128, 1024], f32, tag="ps", name="ps")
                    for f in range(NPAIR):
                        for (a, b) in ((0, 512), (512, 768)):
                            nc.tensor.matmul(
                                out=ps[:, a:b],
                                lhsT=aTb[f][:, st * 128:(st + 1) * 128],
                                rhs=wos[:, f * D + a:f * D + b],
                                start=(f == 0), stop=(f == NPAIR - 1),
                                skip_group_check=True)
                    ot = etp.tile([128, 1024], bf16, tag="et", name="et")
                    nc.vector.tensor_copy(ot[:, 0:D], ps[:, 0:D])
                    nc.sync.dma_start(out=o[st * 128:(st + 1) * 128, :],
                                      in_=ot[:, 0:D])

    nc.finalize()
    return nc


def _mask_tiles():
    import ml_dtypes
    # tri01[jp, q] = 1 iff key jp <= query q (within the diagonal tile)
    tri01 = np.triu(np.ones((128, 128), np.float32))
    # sel broadcasts rec rows (0 -> out 0..63, 64 -> out 64..127)
    sel = np.zeros((128, 128), np.float32)
    sel[0, 0:64] = 1.0
    sel[64, 64:128] = 1.0
    return np.stack([tri01, sel]).astype(ml_dtypes.bfloat16)


def _host_reference(x, mask, w_qkv, w_out):
    qkv = x.astype(np.float64) @ w_qkv.astype(np.float64)
    q, k, v = np.split(qkv, 3, axis=-1)

    def heads(t):
        return t.reshape(B, N, H, DH).transpose(0, 2, 1, 3)
    q, k, v = heads(q), heads(k), heads(v)
    s = np.einsum('bhqd,bhkd->bhqk', q, k) / np.sqrt(DH)
    s = np.where(np.asarray(mask).reshape(1, 1, N, N) == 0, -np.inf, s)
    s = s - s.max(-1, keepdims=True)
    e = np.exp(s)
    p = e / e.sum(-1, keepdims=True)
    out = np.einsum('bhqk,bhkd->bhqd', p, v)
    out = out.transpose(0, 2, 1, 3).reshape(B, N, D)
    return (out @ w_out.astype(np.float64)).astype(np.float32)


def kernel(x, mask, w_qkv, w_out):
    import ml_dtypes
    bf = ml_dtypes.bfloat16
    x = np.asarray(x)
    w_qkv = np.asarray(w_qkv)
    w_out = np.asarray(w_out)

    causal = np.array_equal(
        np.asarray(mask).reshape(N, N) != 0, np.tril(np.ones((N, N), bool)))
    if not causal:
        return _host_reference(x, mask, w_qkv, w_out)

    from concourse.bass_utils import run_bass_kernel_spmd
    if "nc" not in _CACHE:
        _CACHE["nc"] = _build_nc()
    nc = _CACHE["nc"]

    cstn = _mask_tiles()
    W = HH * DH  # 384
    wqk_h, wv_h, wo_h = [], [], []
    for hg in range(2):
        wqk_h.append(np.ascontiguousarray(np.concatenate(
            [w_qkv[:, hg * W:(hg + 1) * W],
             w_qkv[:, D + hg * W:D + (hg + 1) * W]], axis=1)).astype(bf))
        wv_h.append(np.ascontiguousarray(
            w_qkv[:, 2 * D + hg * W:2 * D + (hg + 1) * W]).astype(bf))
        wo_h.append(np.ascontiguousarray(
            w_out[hg * W:(hg + 1) * W, :]).astype(bf))
    xts = [np.ascontiguousarray(x[b].T).astype(bf) for b in range(B)]

    in_maps = []
    for c in range(8):
        b, hg = c // 2, c % 2
        in_maps.append({
            "xt": xts[b],
            "wqk": wqk_h[hg], "wv": wv_h[hg], "wo": wo_h[hg],
            "cst": cstn,
        })
    res = run_bass_kernel_spmd(nc, in_maps, core_ids=list(range(8)),
                               **_CACHE.get("run_kwargs", {}))
    _CACHE["last_res"] = res
    out = np.empty((B, N, D), np.float32)
    for b in range(B):
        out[b] = (res.results[2 * b]["o"].astype(np.float32)
                  + res.results[2 * b + 1]["o"].astype(np.float32))
    return out


# revision 53
# speedup vs baseline: 1.0129x; 1.0129x over previous
"""Causal multi-head attention (B=4, N=2048, D=768, H=12) on 8 TRN2 cores.

Sharding: batch (4) x head-split (2). Core c = (b, hg) handles batch b
and heads 6*hg .. 6*hg+5 over the FULL sequence: QKV projections take
only this half's weight columns, attention runs 6 heads, and the out
projection uses only this half's weight rows, producing a partial
[N, D] that the host sums across the pair of cores sharing a batch.

All matmul operands are bf16 (fp32 PSUM accumulation). Per core:
  qT/kT = Wqk^T x^T  ([2*64, N] head-pair tiles)    v = x Wv (+ones col)
  per (head, query-half, key-tile jt), software-pipelined so the PE
  never waits on the Act engine (scores for jt+1 issue before AV jt):
    s = kT_jt^T qT;  et = exp(s/8) bf16 (* 0/1 causal mask on diag
    tiles, DVE);  oT[65, :] += v_aug_jt^T et  (row 64 = denominators)
  aT = oT[0:64] (pair-packed via DMA repartition for odd heads)
  rec broadcast via PE selector-matmul into PSUM (deferred into the
  next head's loop);  aTb = aT * rec  bf16;   o_partial = aTb^T Wo
"""

import numpy as np

B, N, D, H = 4, 2048, 768, 12
DH = D // H          # 64
HH = H // 2          # 6 local heads per core
NPAIR = HH // 2      # 3 head pairs
KC = D // 128        # 6 contraction chunks
VW = HH * (DH + 1)   # 390 (v_aug row width per seq tile)
NEG = -30000.0

_CACHE = {}


def _build_nc(unused=None):
    import concourse.bacc as bacc
    import concourse.bass as bass
    import concourse.mybir as mybir
    import concourse.tile as tile
    from contextlib import ExitStack

    dt = mybir.dt
    f32 = dt.float32
    bf16 = dt.bfloat16
    fp8 = dt.float8e4
    Exp = mybir.ActivationFunctionType.Exp

    nc = bacc.Bacc(None)
    xt = nc.declare_dram_parameter("xt", [D, N], bf16, isOutput=False)
    wqk = nc.declare_dram_parameter("wqk", [D, 2 * HH * DH], bf16,
                                    isOutput=False)
    wv = nc.declare_dram_parameter("wv", [D, HH * DH], bf16, isOutput=False)
    wo = nc.declare_dram_parameter("wo", [HH * DH, D], bf16, isOutput=False)
    cst = nc.declare_dram_parameter("cst", [2, 128, 128], bf16, isOutput=False)
    o = nc.declare_dram_parameter("o", [N, D], bf16, isOutput=True)

    with tile.TileContext(nc) as tc:
        with ExitStack() as es:
            persist = es.enter_context(tc.tile_pool(name="persist", bufs=1))
            qT = [persist.tile([128, N], bf16, tag=f"qT{m}", name=f"qT{m}")
                  for m in range(NPAIR)]
            kT = [persist.tile([128, N], bf16, tag=f"kT{m}", name=f"kT{m}")
                  for m in range(NPAIR)]
            vp_all = persist.tile([128, 16 * VW], bf16, tag="vp", name="vp")
            vp = [vp_all[:, j * VW:(j + 1) * VW] for j in range(16)]
            aT = [persist.tile([128, N], f32, tag=f"aT{m}", name=f"aT{m}")
                  for m in range(NPAIR)]
            aTb = [persist.tile([128, N], bf16, tag=f"aTb{m}", name=f"aTb{m}")
                   for m in range(NPAIR)]
            # den/rec rows live at partitions 0 (even head) and 64 (odd
            # head) so the PE rec-broadcast lhsT/rhs base partitions align.
            den = [persist.tile([65, N], f32, tag=f"den{m}", name=f"den{m}")
                   for m in range(NPAIR)]
            rec = [persist.tile([65, N], f32, tag=f"rec{m}", name=f"rec{m}")
                   for m in range(NPAIR)]
            recb = persist.tile([128, N], bf16, tag="recb", name="recb")
            for m in range(NPAIR):
                nc.vector.memset(den[m][:], 1.0)
            nc.vector.memset(recb[:], 1.0)
            msk = persist.tile([128, 256], bf16, tag="msk", name="msk")
            tri01 = msk[:, 0:128]
            sel = msk[:, 128:256]
            wos = persist.tile([128, NPAIR * D], bf16, tag="wos", name="wos")

            # ---------------- projections ----------------
            with tc.tile_pool(name="xw", bufs=1) as xwp, \
                 tc.tile_pool(name="on", bufs=1) as onp, \
                 tc.tile_pool(name="pp", bufs=1, space="PSUM") as pp, \
                 tc.tile_pool(name="ppv", bufs=2, space="PSUM") as ppv:
                xts = xwp.tile([128, KC * N], bf16, tag="xts", name="xts")
                wqks = xwp.tile([128, KC * 768], bf16, tag="wqks",
                                name="wqks")
                wvs = xwp.tile([128, KC * 384], bf16, tag="wvs", name="wvs")
                for k in range(KC):
                    nc.scalar.dma_start(out=wqks[:, k * 768:(k + 1) * 768],
                                        in_=wqk[k * 128:(k + 1) * 128, :])
                    nc.sync.dma_start(
                        out=xts[:, k * N:k * N + 512],
                        in_=xt[k * 128:(k + 1) * 128, 0:512])

                # q/k projection, k-major: 6 output tiles accumulate in
                # PSUM while the k-chunk DMAs stream in behind them
                for qtr in range(4):
                    c0 = qtr * 512
                    if qtr > 0:
                        for k in range(KC):
                            nc.sync.dma_start(
                                out=xts[:, k * N + c0:k * N + c0 + 512],
                                in_=xt[k * 128:(k + 1) * 128, c0:c0 + 512])
                    pss = [pp.tile([128, 512], f32, tag=f"psq{m}",
                                   name=f"psq{m}") for m in range(2 * NPAIR)]
                    for k in range(KC):
                        for m in range(2 * NPAIR):
                            nc.tensor.matmul(
                                out=pss[m][:],
                                lhsT=wqks[:, k * 768 + m * 128:
                                          k * 768 + (m + 1) * 128],
                                rhs=xts[:, k * N + c0:k * N + c0 + 512],
                                start=(k == 0), stop=(k == KC - 1))
                    for m in range(2 * NPAIR):
                        dst = qT[m] if m < NPAIR else kT[m - NPAIR]
                        nc.vector.tensor_copy(dst[:, c0:c0 + 512],
                                              pss[m][:])
                for k in range(KC):
                    nc.gpsimd.dma_start(out=wvs[:, k * 384:(k + 1) * 384],
                                        in_=wv[k * 128:(k + 1) * 128, :])
                for i2 in range(2):
                    nc.sync.dma_start(out=msk[:, i2 * 128:(i2 + 1) * 128],
                                      in_=cst[i2])
                for f in range(NPAIR):
                    nc.gpsimd.dma_start(out=wos[:, f * D:(f + 1) * D],
                                        in_=wo[f * 128:(f + 1) * 128, :])

                # v projection: out [seq 128, 6*64] per seq tile
                for st in range(16):
                    ps = ppv.tile([128, 384], f32, tag="psv", name="psv")
                    for k in range(KC):
                        nc.tensor.matmul(
                            out=ps[:],
                            lhsT=xts[:, k * N + st * 128:
                                     k * N + (st + 1) * 128],
                            rhs=wvs[:, k * 384:(k + 1) * 384],
                            start=(k == 0), stop=(k == KC - 1))
                    vv = vp[st].rearrange("p (h c) -> p h c", c=DH + 1)
                    nc.vector.tensor_copy(
                        vv[:, :, 0:DH],
                        ps[:].rearrange("p (h c) -> p h c", c=DH))
                ones96 = onp.tile([128, 16 * HH], bf16, tag="on",
                                  name="ones96")
                nc.vector.memset(ones96[:], 1.0)
                vview = vp_all.rearrange("p (j c) -> p j c", c=DH + 1)
                nc.vector.tensor_copy(
                    vview[:, :, DH:DH + 1],
                    ones96[:].rearrange("p (a b) -> p a b", b=1))

            # ---------------- attention ----------------
            import concourse.bass as bass  # noqa: F401

            def r32(ap):
                return ap.bitcast(dt.float32r)

            with tc.tile_pool(name="et", bufs=5) as etp, \
                 tc.tile_pool(name="dtm", bufs=2) as dtp, \
                 tc.tile_pool(name="omp", bufs=2) as omp, \
                 tc.tile_pool(name="ps4", bufs=2, space="PSUM") as ps4, \
                 tc.tile_pool(name="po4", bufs=2, space="PSUM") as po4:
                # normalize PE-work (rec broadcast + multiply) is deferred
                # into a later jt loop so the PE never waits on the
                # DVE/DMA den->rec chain
                pnorm = []

                def flush_norm(pool):
                    while pnorm:
                        m_, q0_ = pnorm.pop(0)
                        rbps = pool.tile([128, 1024], f32, tag="ps",
                                         name="ps")
                        for (a, b) in ((0, 512), (512, 1024)):
                            nc.tensor.matmul(
                                out=rbps[:, a:b], lhsT=sel,
                                rhs=recb[:, q0_ + a:q0_ + b],
                                start=True, stop=True,
                                skip_group_check=True)
                        nc.vector.tensor_mul(
                            aTb[m_][:, q0_:q0_ + 1024],
                            aT[m_][:, q0_:q0_ + 1024], rbps[:])

                for m in range(NPAIR):
                    otmp = omp.tile([64, N], f32, tag="otmp", name="otmp")
                    for hh in range(2):
                        h = 2 * m + hh
                        hs = slice(hh * 64, hh * 64 + 64)
                        for qc in range(2):
                            q0 = qc * 1024
                            oT = po4.tile([65, 1024], f32, tag="oT",
                                          name="oT")
                            njt = 8 * qc + 8

                            def do_av(jt, et, regs):
                                for (a, b) in regs:
                                    nc.tensor.matmul(
                                        out=oT[:, a:b],
                                        lhsT=vp[jt][:, h * (DH + 1):
                                                    (h + 1) * (DH + 1)],
                                        rhs=et[:, a:b],
                                        start=(jt == 0),
                                        stop=(jt == (8 * qc + 3 if b <= 512
                                                     else njt - 1)),
                                        skip_group_check=True)

                            # jt-pair bursting: S,S then the previous
                            # pair's AV,AV — fewer stationary switches and
                            # semaphore-gated PE instructions per step
                            pend = []
                            for jt in range(njt):
                                ql0 = max(0, 128 * jt - q0)
                                diag = jt >= 8 * qc
                                regs = []
                                if ql0 < 512:
                                    regs.append((ql0, 512))
                                regs.append((max(ql0, 512), 1024))
                                if jt == 6:
                                    flush_norm(ps4)
                                ps = ps4.tile([128, 1024], f32, tag="ps",
                                              name="ps")
                                for (a, b) in regs:
                                    nc.tensor.matmul(
                                        out=ps[:, a:b],
                                        lhsT=kT[m][hs, jt * 128:
                                                   (jt + 1) * 128],
                                        rhs=qT[m][hs, q0 + a:q0 + b],
                                        start=True, stop=True,
                                        skip_group_check=True)
                                et = etp.tile([128, 1024], bf16, tag="et",
                                              name="et")
                                nc.scalar.activation(
                                    out=et[:, ql0:1024], in_=ps[:, ql0:1024],
                                    func=Exp, scale=0.125)
                                if diag:
                                    nc.vector.tensor_mul(
                                        et[:, ql0:ql0 + 128],
                                        et[:, ql0:ql0 + 128], tri01)
                                pend.append((jt, et, regs))
                                if jt % 2 == 1 and len(pend) == 4:
                                    do_av(*pend.pop(0))
                                    do_av(*pend.pop(0))
                            for p_ in pend:
                                do_av(*p_)
                            # drain oT: rows 0..63 -> aT / otmp, row 64 -> den
                            if hh == 0:
                                nc.vector.tensor_copy(
                                    aT[m][0:64, q0:q0 + 1024], oT[0:64, :])
                            else:
                                nc.vector.tensor_copy(
                                    otmp[0:64, q0:q0 + 1024], oT[0:64, :])
                            dtm = dtp.tile([65, 1024], f32, tag="dtm",
                                           name="dtm")
                            nc.vector.tensor_copy(dtm[64:65, :], oT[64:65, :])
                            dr = hh * 64  # den row: partition 0 / 64
                            nc.sync.dma_start(
                                out=den[m][dr:dr + 1, q0:q0 + 1024],
                                in_=dtm[64:65, :])
                            if hh == 1:
                                # pair half complete: repartition odd head,
                                # then normalize this query-half
                                nc.sync.dma_start(
                                    out=aT[m][64:128, q0:q0 + 1024],
                                    in_=otmp[:, q0:q0 + 1024])
                                nc.vector.reciprocal_approx_fast(
                                    out=rec[m][:, q0:q0 + 1024],
                                    in_=den[m][:, q0:q0 + 1024])
                                nc.vector.tensor_copy(
                                    recb[0:65, q0:q0 + 1024],
                                    rec[m][:, q0:q0 + 1024])
                                pnorm.append((m, q0))

                # ------------- output projection -------------
                # runs inside the attention pools (ps/et tiles reused)
                # so no pool-close barrier stalls the PE at the seam
                for st in range(16):
                    if st == 1:
                        # last pair's deferred normalize: its PE ops land
                        # here so st=0's matmuls cover the DVE chain; only
                        # st>=8 reads the columns this mul produces
                        flush_norm(ps4)
                    ps = ps4.tile([128, 1024], f32, tag="ps", name="ps")
                    for f in range(NPAIR):
                        for (a, b) in ((0, 512), (512, 768)):
                            nc.tensor.matmul(
                                out=ps[:, a:b],
                                lhsT=aTb[f][:, st * 128:(st + 1) * 128],
                                rhs=wos[:, f * D + a:f * D + b],
                                start=(f == 0), stop=(f == NPAIR - 1),
                                skip_group_check=True)
                    ot = etp.tile([128, 1024], bf16, tag="et", name="et")
                    nc.vector.tensor_copy(ot[:, 0:D], ps[:, 0:D])
                    nc.sync.dma_start(out=o[st * 128:(st + 1) * 128, :],
                                      in_=ot[:, 0:D])

    nc.finalize()
    return nc


def _mask_tiles():
    import ml_dtypes
    # tri01[jp, q] = 1 iff key jp <= query q (within the diagonal tile)
    tri01 = np.triu(np.ones((128, 128), np.float32))
    # sel broadcasts rec rows (0 -> out 0..63, 64 -> out 64..127)
    sel = np.zeros((128, 128), np.float32)
    sel[0, 0:64] = 1.0
    sel[64, 64:128] = 1.0
    return np.stack([tri01, sel]).astype(ml_dtypes.bfloat16)


def _host_reference(x, mask, w_qkv, w_out):
    qkv = x.astype(np.float64) @ w_qkv.astype(np.float64)
    q, k, v = np.split(qkv, 3, axis=-1)

    def heads(t):
        return t.reshape(B, N, H, DH).transpose(0, 2, 1, 3)
    q, k, v = heads(q), heads(k), heads(v)
    s = np.einsum('bhqd,bhkd->bhqk', q, k) / np.sqrt(DH)
    s = np.where(np.asarray(mask).reshape(1, 1, N, N) == 0, -np.inf, s)
    s = s - s.max(-1, keepdims=True)
    e = np.exp(s)
    p = e / e.sum(-1, keepdims=True)
    out = np.einsum('bhqk,bhkd->bhqd', p, v)
    out = out.transpose(0, 2, 1, 3).reshape(B, N, D)
    return (out @ w_out.astype(np.float64)).astype(np.float32)


def kernel(x, mask, w_qkv, w_out):
    import ml_dtypes
    bf = ml_dtypes.bfloat16
    x = np.asarray(x)
    w_qkv = np.asarray(w_qkv)
    w_out = np.asarray(w_out)

    causal = np.array_equal(
        np.asarray(mask).reshape(N, N) != 0, np.tril(np.ones((N, N), bool)))
    if not causal:
        return _host_reference(x, mask, w_qkv, w_out)

    from concourse.bass_utils import run_bass_kernel_spmd
    if "nc" not in _CACHE:
        _CACHE["nc"] = _build_nc()
    nc = _CACHE["nc"]

    cstn = _mask_tiles()
    W = HH * DH  # 384
    wqk_h, wv_h, wo_h = [], [], []
    for hg in range(2):
        wqk_h.append(np.ascontiguousarray(np.concatenate(
            [w_qkv[:, hg * W:(hg + 1) * W],
             w_qkv[:, D + hg * W:D + (hg + 1) * W]], axis=1)).astype(bf))
        wv_h.append(np.ascontiguousarray(
            w_qkv[:, 2 * D + hg * W:2 * D + (hg + 1) * W]).astype(bf))
        wo_h.append(np.ascontiguousarray(
            w_out[hg * W:(hg + 1) * W, :]).astype(bf))
    xts = [np.ascontiguousarray(x[b].T).astype(bf) for b in range(B)]

    in_maps = []
    for c in range(8):
        b, hg = c // 2, c % 2
        in_maps.append({
            "xt": xts[b],
            "wqk": wqk_h[hg], "wv": wv_h[hg], "wo": wo_h[hg],
            "cst": cstn,
        })
    res = run_bass_kernel_spmd(nc, in_maps, core_ids=list(range(8)),
                               **_CACHE.get("run_kwargs", {}))
    _CACHE["last_res"] = res
    out = np.empty((B, N, D), np.float32)
    for b in range(B):
        out[b] = (res.results[2 * b]["o"].astype(np.float32)
                  + res.results[2 * b + 1]["o"].astype(np.float32))
    return out


# revision 54
# speedup vs baseline: 1.0316x; 1.0185x over previous
"""Causal multi-head attention (B=4, N=2048, D=768, H=12) on 8 TRN2 cores.

Sharding: batch (4) x head-split (2). Core c = (b, hg) handles batch b
and heads 6*hg .. 6*hg+5 over the FULL sequence: QKV projections take
only this half's weight columns, attention runs 6 heads, and the out
projection uses only this half's weight rows, producing a partial
[N, D] that the host sums across the pair of cores sharing a batch.

All matmul operands are bf16 (fp32 PSUM accumulation). Per core:
  qT/kT = Wqk^T x^T  ([2*64, N] head-pair tiles)    v = x Wv (+ones col)
  per (head, query-half, key-tile jt), software-pipelined so the PE
  never waits on the Act engine (scores for jt+1 issue before AV jt):
    s = kT_jt^T qT;  et = exp(s/8) bf16 (* 0/1 causal mask on diag
    tiles, DVE);  oT[65, :] += v_aug_jt^T et  (row 64 = denominators)
  aT = oT[0:64] (pair-packed via DMA repartition for odd heads)
  rec broadcast via PE selector-matmul into PSUM (deferred into the
  next head's loop);  aTb = aT * rec  bf16;   o_partial = aTb^T Wo
"""

import numpy as np

B, N, D, H = 4, 2048, 768, 12
DH = D // H          # 64
HH = H // 2          # 6 local heads per core
NPAIR = HH // 2      # 3 head pairs
KC = D // 128        # 6 contraction chunks
VW = HH * (DH + 1)   # 390 (v_aug row width per seq tile)
NEG = -30000.0

_CACHE = {}


def _build_nc(unused=None):
    import concourse.bacc as bacc
    import concourse.bass as bass
    import concourse.mybir as mybir
    import concourse.tile as tile
    from contextlib import ExitStack

    dt = mybir.dt
    f32 = dt.float32
    bf16 = dt.bfloat16
    fp8 = dt.float8e4
    Exp = mybir.ActivationFunctionType.Exp

    nc = bacc.Bacc(None)
    xt = nc.declare_dram_parameter("xt", [D, N], bf16, isOutput=False)
    wqk = nc.declare_dram_parameter("wqk", [D, 2 * HH * DH], bf16,
                                    isOutput=False)
    wv = nc.declare_dram_parameter("wv", [D, HH * DH], bf16, isOutput=False)
    wo = nc.declare_dram_parameter("wo", [HH * DH, D], bf16, isOutput=False)
    cst = nc.declare_dram_parameter("cst", [2, 128, 128], bf16, isOutput=False)
    o = nc.declare_dram_parameter("o", [N, D], bf16, isOutput=True)

    with tile.TileContext(nc) as tc:
        with ExitStack() as es:
            persist = es.enter_context(tc.tile_pool(name="persist", bufs=1))
            qT = [persist.tile([128, N], bf16, tag=f"qT{m}", name=f"qT{m}")
                  for m in range(NPAIR)]
            kT = [persist.tile([128, N], bf16, tag=f"kT{m}", name=f"kT{m}")
                  for m in range(NPAIR)]
            vp_all = persist.tile([128, 16 * VW], bf16, tag="vp", name="vp")
            vp = [vp_all[:, j * VW:(j + 1) * VW] for j in range(16)]
            aT = [persist.tile([128, N], f32, tag=f"aT{m}", name=f"aT{m}")
                  for m in range(NPAIR)]
            aTb = [persist.tile([128, N], bf16, tag=f"aTb{m}", name=f"aTb{m}")
                   for m in range(NPAIR)]
            # den/rec rows live at partitions 0 (even head) and 64 (odd
            # head) so the PE rec-broadcast lhsT/rhs base partitions align.
            den = [persist.tile([65, N], f32, tag=f"den{m}", name=f"den{m}")
                   for m in range(NPAIR)]
            rec = [persist.tile([65, N], f32, tag=f"rec{m}", name=f"rec{m}")
                   for m in range(NPAIR)]
            recb = persist.tile([128, N], bf16, tag="recb", name="recb")
            for m in range(NPAIR):
                nc.vector.memset(den[m][:], 1.0)
            nc.vector.memset(recb[:], 1.0)
            msk = persist.tile([128, 256], bf16, tag="msk", name="msk")
            tri01 = msk[:, 0:128]
            sel = msk[:, 128:256]
            wos = persist.tile([128, NPAIR * D], bf16, tag="wos", name="wos")

            # ---------------- projections ----------------
            with tc.tile_pool(name="xw", bufs=1) as xwp, \
                 tc.tile_pool(name="on", bufs=1) as onp, \
                 tc.tile_pool(name="pp", bufs=1, space="PSUM") as pp, \
                 tc.tile_pool(name="ppv", bufs=2, space="PSUM") as ppv:
                xts = xwp.tile([128, KC * N], bf16, tag="xts", name="xts")
                wqks = xwp.tile([128, KC * 768], bf16, tag="wqks",
                                name="wqks")
                wvs = xwp.tile([128, KC * 384], bf16, tag="wvs", name="wvs")
                for k in range(KC):
                    nc.scalar.dma_start(out=wqks[:, k * 768:(k + 1) * 768],
                                        in_=wqk[k * 128:(k + 1) * 128, :])
                    nc.sync.dma_start(
                        out=xts[:, k * N:k * N + 512],
                        in_=xt[k * 128:(k + 1) * 128, 0:512])

                # q/k projection, k-major: 6 output tiles accumulate in
                # PSUM while the k-chunk DMAs stream in behind them
                for qtr in range(4):
                    c0 = qtr * 512
                    if qtr > 0:
                        for k in range(KC):
                            nc.sync.dma_start(
                                out=xts[:, k * N + c0:k * N + c0 + 512],
                                in_=xt[k * 128:(k + 1) * 128, c0:c0 + 512])
                    pss = [pp.tile([128, 512], f32, tag=f"psq{m}",
                                   name=f"psq{m}") for m in range(2 * NPAIR)]
                    for k in range(KC):
                        for m in range(2 * NPAIR):
                            nc.tensor.matmul(
                                out=pss[m][:],
                                lhsT=wqks[:, k * 768 + m * 128:
                                          k * 768 + (m + 1) * 128],
                                rhs=xts[:, k * N + c0:k * N + c0 + 512],
                                start=(k == 0), stop=(k == KC - 1))
                    for m in range(2 * NPAIR):
                        dst = qT[m] if m < NPAIR else kT[m - NPAIR]
                        nc.vector.tensor_copy(dst[:, c0:c0 + 512],
                                              pss[m][:])
                for k in range(KC):
                    nc.gpsimd.dma_start(out=wvs[:, k * 384:(k + 1) * 384],
                                        in_=wv[k * 128:(k + 1) * 128, :])
                for i2 in range(2):
                    nc.sync.dma_start(out=msk[:, i2 * 128:(i2 + 1) * 128],
                                      in_=cst[i2])
                for f in range(NPAIR):
                    nc.gpsimd.dma_start(out=wos[:, f * D:(f + 1) * D],
                                        in_=wo[f * 128:(f + 1) * 128, :])

                # v projection: out [seq 128, 6*64] per seq tile
                for st in range(16):
                    ps = ppv.tile([128, 384], f32, tag="psv", name="psv")
                    for k in range(KC):
                        nc.tensor.matmul(
                            out=ps[:],
                            lhsT=xts[:, k * N + st * 128:
                                     k * N + (st + 1) * 128],
                            rhs=wvs[:, k * 384:(k + 1) * 384],
                            start=(k == 0), stop=(k == KC - 1))
                    vv = vp[st].rearrange("p (h c) -> p h c", c=DH + 1)
                    nc.vector.tensor_copy(
                        vv[:, :, 0:DH],
                        ps[:].rearrange("p (h c) -> p h c", c=DH))
                ones96 = onp.tile([128, 16 * HH], bf16, tag="on",
                                  name="ones96")
                nc.vector.memset(ones96[:], 1.0)
                vview = vp_all.rearrange("p (j c) -> p j c", c=DH + 1)
                nc.vector.tensor_copy(
                    vview[:, :, DH:DH + 1],
                    ones96[:].rearrange("p (a b) -> p a b", b=1))

            # ---------------- attention ----------------
            import concourse.bass as bass  # noqa: F401

            def r32(ap):
                return ap.bitcast(dt.float32r)

            with tc.tile_pool(name="et", bufs=5) as etp, \
                 tc.tile_pool(name="dtm", bufs=2) as dtp, \
                 tc.tile_pool(name="omp", bufs=2) as omp, \
                 tc.tile_pool(name="ps4", bufs=2, space="PSUM") as ps4, \
                 tc.tile_pool(name="po4", bufs=2, space="PSUM") as po4:
                # normalize PE-work (rec broadcast + multiply) is deferred
                # into a later jt loop so the PE never waits on the
                # DVE/DMA den->rec chain
                pnorm = []

                def flush_norm(pool):
                    while pnorm:
                        m_, q0_ = pnorm.pop(0)
                        rbps = pool.tile([128, 1024], f32, tag="ps",
                                         name="ps")
                        for (a, b) in ((0, 512), (512, 1024)):
                            nc.tensor.matmul(
                                out=rbps[:, a:b], lhsT=sel,
                                rhs=recb[:, q0_ + a:q0_ + b],
                                start=True, stop=True,
                                skip_group_check=True)
                        nc.vector.tensor_mul(
                            aTb[m_][:, q0_:q0_ + 1024],
                            aT[m_][:, q0_:q0_ + 1024], rbps[:])

                for m in range(NPAIR):
                    otmp = omp.tile([64, N], f32, tag="otmp", name="otmp")
                    for hh in range(2):
                        h = 2 * m + hh
                        hs = slice(hh * 64, hh * 64 + 64)
                        for qc in range(2):
                            q0 = qc * 1024
                            oT = po4.tile([65, 1024], f32, tag="oT",
                                          name="oT")
                            njt = 8 * qc + 8

                            def do_av(jt, et, regs):
                                for (a, b) in regs:
                                    nc.tensor.matmul(
                                        out=oT[:, a:b],
                                        lhsT=vp[jt][:, h * (DH + 1):
                                                    (h + 1) * (DH + 1)],
                                        rhs=et[:, a:b],
                                        start=(jt == 0),
                                        stop=(jt == (8 * qc + 3 if b <= 512
                                                     else njt - 1)),
                                        skip_group_check=True)

                            # jt-pair bursting: S,S then the previous
                            # pair's AV,AV — fewer stationary switches and
                            # semaphore-gated PE instructions per step
                            pend = []
                            for jt in range(njt):
                                ql0 = max(0, 128 * jt - q0)
                                diag = jt >= 8 * qc
                                regs = []
                                if ql0 < 512:
                                    regs.append((ql0, 512))
                                regs.append((max(ql0, 512), 1024))
                                if jt == 6:
                                    flush_norm(ps4)
                                ps = ps4.tile([128, 1024], f32, tag="ps",
                                              name="ps")
                                for (a, b) in regs:
                                    nc.tensor.matmul(
                                        out=ps[:, a:b],
                                        lhsT=kT[m][hs, jt * 128:
                                                   (jt + 1) * 128],
                                        rhs=qT[m][hs, q0 + a:q0 + b],
                                        start=True, stop=True,
                                        skip_group_check=True)
                                et = etp.tile([128, 1024], bf16, tag="et",
                                              name="et")
                                nc.scalar.activation(
                                    out=et[:, ql0:1024], in_=ps[:, ql0:1024],
                                    func=Exp, scale=0.125)
                                if diag:
                                    nc.vector.tensor_mul(
                                        et[:, ql0:ql0 + 128],
                                        et[:, ql0:ql0 + 128], tri01)
                                pend.append((jt, et, regs))
                                if jt % 2 == 1 and len(pend) == 4:
                                    do_av(*pend.pop(0))
                                    do_av(*pend.pop(0))
                            for p_ in pend:
                                do_av(*p_)
                            # drain oT: rows 0..63 -> aT / otmp, row 64 -> den
                            if hh == 0:
                                nc.vector.tensor_copy(
                                    aT[m][0:64, q0:q0 + 1024], oT[0:64, :])
                            else:
                                nc.vector.tensor_copy(
                                    otmp[0:64, q0:q0 + 1024], oT[0:64, :])
                            dtm = dtp.tile([65, 1024], f32, tag="dtm",
                                           name="dtm")
                            nc.vector.tensor_copy(dtm[64:65, :], oT[64:65, :])
                            dr = hh * 64  # den row: partition 0 / 64
                            nc.sync.dma_start(
                                out=den[m][dr:dr + 1, q0:q0 + 1024],
                                in_=dtm[64:65, :])
                            if hh == 1:
                                # pair half complete: repartition odd head,
                                # then normalize this query-half
                                nc.sync.dma_start(
                                    out=aT[m][64:128, q0:q0 + 1024],
                                    in_=otmp[:, q0:q0 + 1024])
                                nc.vector.reciprocal_approx_fast(
                                    out=rec[m][:, q0:q0 + 1024],
                                    in_=den[m][:, q0:q0 + 1024])
                                nc.vector.tensor_copy(
                                    recb[0:65, q0:q0 + 1024],
                                    rec[m][:, q0:q0 + 1024])
                                pnorm.append((m, q0))

            # ---------------- output projection ----------------
            with tc.tile_pool(name="osb", bufs=3) as osb, \
                 tc.tile_pool(name="rbf", bufs=1, space="PSUM") as rbf, \
                 tc.tile_pool(name="pp5", bufs=3, space="PSUM") as pp5:
                for st in range(16):
                    if st == 1:
                        # last pair's deferred normalize: its PE ops land
                        # here so st=0's matmuls cover the DVE chain; only
                        # st>=8 reads the columns this mul produces
                        flush_norm(rbf)
                    ps = pp5.tile([128, D], f32, tag="ps5", name="ps5")
                    for f in range(NPAIR):
                        for (a, b) in ((0, 512), (512, 768)):
                            nc.tensor.matmul(
                                out=ps[:, a:b],
                                lhsT=aTb[f][:, st * 128:(st + 1) * 128],
                                rhs=wos[:, f * D + a:f * D + b],
                                start=(f == 0), stop=(f == NPAIR - 1))
                    ot = osb.tile([128, D], bf16, tag="ot", name="ot")
                    nc.vector.tensor_copy(ot[:], ps[:])
                    nc.sync.dma_start(out=o[st * 128:(st + 1) * 128, :],
                                      in_=ot[:])

    nc.finalize()
    return nc


def _mask_tiles():
    import ml_dtypes
    # tri01[jp, q] = 1 iff key jp <= query q (within the diagonal tile)
    tri01 = np.triu(np.ones((128, 128), np.float32))
    # sel broadcasts rec rows (0 -> out 0..63, 64 -> out 64..127)
    sel = np.zeros((128, 128), np.float32)
    sel[0, 0:64] = 1.0
    sel[64, 64:128] = 1.0
    return np.stack([tri01, sel]).astype(ml_dtypes.bfloat16)


def _host_reference(x, mask, w_qkv, w_out):
    qkv = x.astype(np.float64) @ w_qkv.astype(np.float64)
    q, k, v = np.split(qkv, 3, axis=-1)

    def heads(t):
        return t.reshape(B, N, H, DH).transpose(0, 2, 1, 3)
    q, k, v = heads(q), heads(k), heads(v)
    s = np.einsum('bhqd,bhkd->bhqk', q, k) / np.sqrt(DH)
    s = np.where(np.asarray(mask).reshape(1, 1, N, N) == 0, -np.inf, s)
    s = s - s.max(-1, keepdims=True)
    e = np.exp(s)
    p = e / e.sum(-1, keepdims=True)
    out = np.einsum('bhqk,bhkd->bhqd', p, v)
    out = out.transpose(0, 2, 1, 3).reshape(B, N, D)
    return (out @ w_out.astype(np.float64)).astype(np.float32)


def kernel(x, mask, w_qkv, w_out):
    import ml_dtypes
    bf = ml_dtypes.bfloat16
    x = np.asarray(x)
    w_qkv = np.asarray(w_qkv)
    w_out = np.asarray(w_out)

    causal = np.array_equal(
        np.asarray(mask).reshape(N, N) != 0, np.tril(np.ones((N, N), bool)))
    if not causal:
        return _host_reference(x, mask, w_qkv, w_out)

    from concourse.bass_utils import run_bass_kernel_spmd
    if "nc" not in _CACHE:
        _CACHE["nc"] = _build_nc()
    nc = _CACHE["nc"]

    cstn = _mask_tiles()
    W = HH * DH  # 384
    wqk_h, wv_h, wo_h = [], [], []
    for hg in range(2):
        wqk_h.append(np.ascontiguousarray(np.concatenate(
            [w_qkv[:, hg * W:(hg + 1) * W],
             w_qkv[:, D + hg * W:D + (hg + 1) * W]], axis=1)).astype(bf))
        wv_h.append(np.ascontiguousarray(
            w_qkv[:, 2 * D + hg * W:2 * D + (hg + 1) * W]).astype(bf))
        wo_h.append(np.ascontiguousarray(
            w_out[hg * W:(hg + 1) * W, :]).astype(bf))
    xts = [np.ascontiguousarray(x[b].T).astype(bf) for b in range(B)]

    in_maps = []
    for c in range(8):
        b, hg = c // 2, c % 2
        in_maps.append({
            "xt": xts[b],
            "wqk": wqk_h[hg], "wv": wv_h[hg], "wo": wo_h[hg],
            "cst": cstn,
        })
    res = run_bass_kernel_spmd(nc, in_maps, core_ids=list(range(8)),
                               **_CACHE.get("run_kwargs", {}))
    _CACHE["last_res"] = res
    out = np.empty((B, N, D), np.float32)
    for b in range(B):
        out[b] = (res.results[2 * b]["o"].astype(np.float32)
                  + res.results[2 * b + 1]["o"].astype(np.float32))
    return out


# revision 56
# speedup vs baseline: 1.1476x; 1.1124x over previous
"""Causal multi-head attention (B=4, N=2048, D=768, H=12) on 8 TRN2 cores.

Sharding: batch (4) x head-split (2). Core c = (b, hg) handles batch b
and heads 6*hg .. 6*hg+5 over the FULL sequence: QKV projections take
only this half's weight columns, attention runs 6 heads, and the out
projection uses only this half's weight rows, producing a partial
[N, D] that the host sums across the pair of cores sharing a batch.

All matmul operands are bf16 (fp32 PSUM accumulation). Per core:
  qT/kT = Wqk^T x^T  ([2*64, N] head-pair tiles)    v = x Wv (+ones col)
  per (head, query-half, key-tile jt), software-pipelined so the PE
  never waits on the Act engine (scores for jt+1 issue before AV jt):
    s = kT_jt^T qT;  et = exp(s/8) bf16 (* 0/1 causal mask on diag
    tiles, DVE);  oT[65, :] += v_aug_jt^T et  (row 64 = denominators)
  aT = oT[0:64] (pair-packed via DMA repartition for odd heads)
  rec broadcast via PE selector-matmul into PSUM (deferred into the
  next head's loop);  aTb = aT * rec  bf16;   o_partial = aTb^T Wo
"""

import numpy as np

B, N, D, H = 4, 2048, 768, 12
DH = D // H          # 64
HH = H // 2          # 6 local heads per core
NPAIR = HH // 2      # 3 head pairs
KC = D // 128        # 6 contraction chunks
VW = HH * (DH + 1)   # 390 (v_aug row width per seq tile)
NEG = -30000.0

_CACHE = {}


def _build_nc(unused=None):
    import concourse.bacc as bacc
    import concourse.bass as bass
    import concourse.mybir as mybir
    import concourse.tile as tile
    from contextlib import ExitStack

    dt = mybir.dt
    f32 = dt.float32
    bf16 = dt.bfloat16
    fp8 = dt.float8e4
    Exp = mybir.ActivationFunctionType.Exp

    nc = bacc.Bacc(None)
    xt = nc.declare_dram_parameter("xt", [D, N], bf16, isOutput=False)
    wqk = nc.declare_dram_parameter("wqk", [D, 2 * HH * DH], bf16,
                                    isOutput=False)
    wv = nc.declare_dram_parameter("wv", [D, HH * DH], bf16, isOutput=False)
    wo = nc.declare_dram_parameter("wo", [HH * DH, D], bf16, isOutput=False)
    cst = nc.declare_dram_parameter("cst", [2, 128, 128], bf16, isOutput=False)
    o = nc.declare_dram_parameter("o", [N, D], bf16, isOutput=True)

    with tile.TileContext(nc) as tc:
        with ExitStack() as es:
            persist = es.enter_context(tc.tile_pool(name="persist", bufs=1))
            qT = [persist.tile([128, N], bf16, tag=f"qT{m}", name=f"qT{m}")
                  for m in range(NPAIR)]
            kT = [persist.tile([128, N], bf16, tag=f"kT{m}", name=f"kT{m}")
                  for m in range(NPAIR)]
            vp_all = persist.tile([128, 16 * VW], bf16, tag="vp", name="vp")
            vp = [vp_all[:, j * VW:(j + 1) * VW] for j in range(16)]
            aT = [persist.tile([128, N], f32, tag=f"aT{m}", name=f"aT{m}")
                  for m in range(NPAIR)]
            aTb = [persist.tile([128, N], bf16, tag=f"aTb{m}", name=f"aTb{m}")
                   for m in range(NPAIR)]
            # den/rec rows live at partitions 0 (even head) and 64 (odd
            # head) so the PE rec-broadcast lhsT/rhs base partitions align.
            den = [persist.tile([65, N], f32, tag=f"den{m}", name=f"den{m}")
                   for m in range(NPAIR)]
            rec = [persist.tile([65, N], f32, tag=f"rec{m}", name=f"rec{m}")
                   for m in range(NPAIR)]
            recb = persist.tile([128, N], bf16, tag="recb", name="recb")
            for m in range(NPAIR):
                nc.vector.memset(den[m][:], 1.0)
            nc.vector.memset(recb[:], 1.0)
            msk = persist.tile([128, 256], bf16, tag="msk", name="msk")
            tri01 = msk[:, 0:128]
            sel = msk[:, 128:256]
            wos = persist.tile([128, NPAIR * D], bf16, tag="wos", name="wos")

            # ---------------- projections ----------------
            with tc.tile_pool(name="xw", bufs=1) as xwp, \
                 tc.tile_pool(name="on", bufs=1) as onp, \
                 tc.tile_pool(name="pp", bufs=1, space="PSUM") as pp, \
                 tc.tile_pool(name="ppv", bufs=2, space="PSUM") as ppv:
                xts = xwp.tile([128, KC * N], bf16, tag="xts", name="xts")
                wqks = xwp.tile([128, KC * 768], bf16, tag="wqks",
                                name="wqks")
                wvs = xwp.tile([128, KC * 384], bf16, tag="wvs", name="wvs")
                for k in range(KC):
                    nc.scalar.dma_start(out=wqks[:, k * 768:(k + 1) * 768],
                                        in_=wqk[k * 128:(k + 1) * 128, :])
                    nc.sync.dma_start(
                        out=xts[:, k * N:k * N + 512],
                        in_=xt[k * 128:(k + 1) * 128, 0:512])

                # q/k projection, k-major: 6 output tiles accumulate in
                # PSUM while the k-chunk DMAs stream in behind them
                for qtr in range(4):
                    c0 = qtr * 512
                    if qtr > 0:
                        for k in range(KC):
                            nc.sync.dma_start(
                                out=xts[:, k * N + c0:k * N + c0 + 512],
                                in_=xt[k * 128:(k + 1) * 128, c0:c0 + 512])
                    pss = [pp.tile([128, 512], f32, tag=f"psq{m}",
                                   name=f"psq{m}") for m in range(2 * NPAIR)]
                    for k in range(KC):
                        for m in range(2 * NPAIR):
                            nc.tensor.matmul(
                                out=pss[m][:],
                                lhsT=wqks[:, k * 768 + m * 128:
                                          k * 768 + (m + 1) * 128],
                                rhs=xts[:, k * N + c0:k * N + c0 + 512],
                                start=(k == 0), stop=(k == KC - 1))
                    for m in range(2 * NPAIR):
                        dst = qT[m] if m < NPAIR else kT[m - NPAIR]
                        nc.vector.tensor_copy(dst[:, c0:c0 + 512],
                                              pss[m][:])
                for k in range(KC):
                    nc.gpsimd.dma_start(out=wvs[:, k * 384:(k + 1) * 384],
                                        in_=wv[k * 128:(k + 1) * 128, :])
                for i2 in range(2):
                    nc.sync.dma_start(out=msk[:, i2 * 128:(i2 + 1) * 128],
                                      in_=cst[i2])
                for f in range(NPAIR):
                    nc.gpsimd.dma_start(out=wos[:, f * D:(f + 1) * D],
                                        in_=wo[f * 128:(f + 1) * 128, :])

                # v projection: out [seq 128, 6*64] per seq tile
                for st in range(16):
                    ps = ppv.tile([128, 384], f32, tag="psv", name="psv")
                    for k in range(KC):
                        nc.tensor.matmul(
                            out=ps[:],
                            lhsT=xts[:, k * N + st * 128:
                                     k * N + (st + 1) * 128],
                            rhs=wvs[:, k * 384:(k + 1) * 384],
                            start=(k == 0), stop=(k == KC - 1))
                    vv = vp[st].rearrange("p (h c) -> p h c", c=DH + 1)
                    nc.vector.tensor_copy(
                        vv[:, :, 0:DH],
                        ps[:].rearrange("p (h c) -> p h c", c=DH))
                ones96 = onp.tile([128, 16 * HH], bf16, tag="on",
                                  name="ones96")
                nc.vector.memset(ones96[:], 1.0)
                vview = vp_all.rearrange("p (j c) -> p j c", c=DH + 1)
                nc.vector.tensor_copy(
                    vview[:, :, DH:DH + 1],
                    ones96[:].rearrange("p (a b) -> p a b", b=1))

            # ---------------- attention ----------------
            import concourse.bass as bass  # noqa: F401

            def r32(ap):
                return ap.bitcast(dt.float32r)

            with tc.tile_pool(name="et", bufs=6) as etp, \
                 tc.tile_pool(name="dtm", bufs=3) as dtp, \
                 tc.tile_pool(name="omp", bufs=2) as omp, \
                 tc.tile_pool(name="ps4", bufs=2, space="PSUM") as ps4, \
                 tc.tile_pool(name="po4", bufs=2, space="PSUM") as po4:
                # normalize PE-work (rec broadcast + multiply) is deferred
                # into a later jt loop so the PE never waits on the
                # DVE/DMA den->rec chain
                pnorm = []

                def flush_norm(pool):
                    while pnorm:
                        m_, q0_ = pnorm.pop(0)
                        rbps = pool.tile([128, 1024], f32, tag="ps",
                                         name="ps")
                        for (a, b) in ((0, 512), (512, 1024)):
                            nc.tensor.matmul(
                                out=rbps[:, a:b], lhsT=sel,
                                rhs=recb[:, q0_ + a:q0_ + b],
                                start=True, stop=True,
                                skip_group_check=True)
                        nc.vector.tensor_mul(
                            aTb[m_][:, q0_:q0_ + 1024],
                            aT[m_][:, q0_:q0_ + 1024], rbps[:])

                for m in range(NPAIR):
                    otmp = omp.tile([64, N], f32, tag="otmp", name="otmp")
                    for hh in range(2):
                        h = 2 * m + hh
                        hs = slice(hh * 64, hh * 64 + 64)
                        for qc in range(2):
                            q0 = qc * 1024
                            oT = po4.tile([65, 1024], f32, tag="oT",
                                          name="oT")
                            njt = 8 * qc + 8

                            def do_av(jt, et, regs):
                                for (a, b) in regs:
                                    nc.tensor.matmul(
                                        out=oT[:, a:b],
                                        lhsT=vp[jt][:, h * (DH + 1):
                                                    (h + 1) * (DH + 1)],
                                        rhs=et[:, a:b],
                                        start=(jt == 0),
                                        stop=(jt == (8 * qc + 3 if b <= 512
                                                     else njt - 1)),
                                        skip_group_check=True)

                            # jt-pair bursting: S,S then the previous
                            # pair's AV,AV — fewer stationary switches and
                            # semaphore-gated PE instructions per step
                            pend = []
                            for jt in range(njt):
                                ql0 = max(0, 128 * jt - q0)
                                diag = jt >= 8 * qc
                                regs = []
                                if ql0 < 512:
                                    regs.append((ql0, 512))
                                regs.append((max(ql0, 512), 1024))
                                if jt == 6:
                                    flush_norm(ps4)
                                ps = ps4.tile([128, 1024], f32, tag="ps",
                                              name="ps")
                                for (a, b) in regs:
                                    nc.tensor.matmul(
                                        out=ps[:, a:b],
                                        lhsT=kT[m][hs, jt * 128:
                                                   (jt + 1) * 128],
                                        rhs=qT[m][hs, q0 + a:q0 + b],
                                        start=True, stop=True,
                                        skip_group_check=True)
                                et = etp.tile([128, 1024], bf16, tag="et",
                                              name="et")
                                nc.scalar.activation(
                                    out=et[:, ql0:1024], in_=ps[:, ql0:1024],
                                    func=Exp, scale=0.125)
                                if diag:
                                    # on the idle Pool engine: keeps the
                                    # DVE queue free for the oT drains and
                                    # the den->rec normalize chain
                                    nc.gpsimd.tensor_mul(
                                        et[:, ql0:ql0 + 128],
                                        et[:, ql0:ql0 + 128], tri01)
                                pend.append((jt, et, regs))
                                if jt % 2 == 1 and len(pend) == 4:
                                    do_av(*pend.pop(0))
                                    do_av(*pend.pop(0))
                            for p_ in pend:
                                do_av(*p_)
                            # drain oT: rows 0..63 -> aT / otmp, row 64 -> den
                            if hh == 0:
                                nc.vector.tensor_copy(
                                    aT[m][0:64, q0:q0 + 1024], oT[0:64, :])
                            else:
                                nc.vector.tensor_copy(
                                    otmp[0:64, q0:q0 + 1024], oT[0:64, :])
                            dtm = dtp.tile([65, 1024], f32, tag="dtm",
                                           name="dtm")
                            nc.vector.tensor_copy(dtm[64:65, :], oT[64:65, :])
                            dr = hh * 64  # den row: partition 0 / 64
                            nc.sync.dma_start(
                                out=den[m][dr:dr + 1, q0:q0 + 1024],
                                in_=dtm[64:65, :])
                            if hh == 1:
                                # pair half complete: repartition odd head,
                                # then normalize this query-half
                                nc.sync.dma_start(
                                    out=aT[m][64:128, q0:q0 + 1024],
                                    in_=otmp[:, q0:q0 + 1024])
                                nc.vector.reciprocal_approx_fast(
                                    out=rec[m][:, q0:q0 + 1024],
                                    in_=den[m][:, q0:q0 + 1024])
                                nc.vector.tensor_copy(
                                    recb[0:65, q0:q0 + 1024],
                                    rec[m][:, q0:q0 + 1024])
                                pnorm.append((m, q0))

            # ---------------- output projection ----------------
            with tc.tile_pool(name="osb", bufs=3) as osb, \
                 tc.tile_pool(name="rbf", bufs=1, space="PSUM") as rbf, \
                 tc.tile_pool(name="pp5", bufs=3, space="PSUM") as pp5:
                for st in range(16):
                    if st == 1:
                        # last pair's deferred normalize: its PE ops land
                        # here so st=0's matmuls cover the DVE chain; only
                        # st>=8 reads the columns this mul produces
                        flush_norm(rbf)
                    ps = pp5.tile([128, D], f32, tag="ps5", name="ps5")
                    for f in range(NPAIR):
                        for (a, b) in ((0, 512), (512, 768)):
                            nc.tensor.matmul(
                                out=ps[:, a:b],
                                lhsT=aTb[f][:, st * 128:(st + 1) * 128],
                                rhs=wos[:, f * D + a:f * D + b],
                                start=(f == 0), stop=(f == NPAIR - 1))
                    ot = osb.tile([128, D], bf16, tag="ot", name="ot")
                    nc.vector.tensor_copy(ot[:], ps[:])
                    nc.sync.dma_start(out=o[st * 128:(st + 1) * 128, :],
                                      in_=ot[:])

    nc.finalize()
    return nc


def _mask_tiles():
    import ml_dtypes
    # tri01[jp, q] = 1 iff key jp <= query q (within the diagonal tile)
    tri01 = np.triu(np.ones((128, 128), np.float32))
    # sel broadcasts rec rows (0 -> out 0..63, 64 -> out 64..127)
    sel = np.zeros((128, 128), np.float32)
    sel[0, 0:64] = 1.0
    sel[64, 64:128] = 1.0
    return np.stack([tri01, sel]).astype(ml_dtypes.bfloat16)


def _host_reference(x, mask, w_qkv, w_out):
    qkv = x.astype(np.float64) @ w_qkv.astype(np.float64)
    q, k, v = np.split(qkv, 3, axis=-1)

    def heads(t):
        return t.reshape(B, N, H, DH).transpose(0, 2, 1, 3)
    q, k, v = heads(q), heads(k), heads(v)
    s = np.einsum('bhqd,bhkd->bhqk', q, k) / np.sqrt(DH)
    s = np.where(np.asarray(mask).reshape(1, 1, N, N) == 0, -np.inf, s)
    s = s - s.max(-1, keepdims=True)
    e = np.exp(s)
    p = e / e.sum(-1, keepdims=True)
    out = np.einsum('bhqk,bhkd->bhqd', p, v)
    out = out.transpose(0, 2, 1, 3).reshape(B, N, D)
    return (out @ w_out.astype(np.float64)).astype(np.float32)


def kernel(x, mask, w_qkv, w_out):
    import ml_dtypes
    bf = ml_dtypes.bfloat16
    x = np.asarray(x)
    w_qkv = np.asarray(w_qkv)
    w_out = np.asarray(w_out)

    causal = np.array_equal(
        np.asarray(mask).reshape(N, N) != 0, np.tril(np.ones((N, N), bool)))
    if not causal:
        return _host_reference(x, mask, w_qkv, w_out)

    from concourse.bass_utils import run_bass_kernel_spmd
    if "nc" not in _CACHE:
        _CACHE["nc"] = _build_nc()
    nc = _CACHE["nc"]

    cstn = _mask_tiles()
    W = HH * DH  # 384
    wqk_h, wv_h, wo_h = [], [], []
    for hg in range(2):
        wqk_h.append(np.ascontiguousarray(np.concatenate(
            [w_qkv[:, hg * W:(hg + 1) * W],
             w_qkv[:, D + hg * W:D + (hg + 1) * W]], axis=1)).astype(bf))
        wv_h.append(np.ascontiguousarray(
            w_qkv[:, 2 * D + hg * W:2 * D + (hg + 1) * W]).astype(bf))
        wo_h.append(np.ascontiguousarray(
            w_out[hg * W:(hg + 1) * W, :]).astype(bf))
    xts = [np.ascontiguousarray(x[b].T).astype(bf) for b in range(B)]

    in_maps = []
    for c in range(8):
        b, hg = c // 2, c % 2
        in_maps.append({
            "xt": xts[b],
            "wqk": wqk_h[hg], "wv": wv_h[hg], "wo": wo_h[hg],
            "cst": cstn,
        })
    res = run_bass_kernel_spmd(nc, in_maps, core_ids=list(range(8)),
                               **_CACHE.get("run_kwargs", {}))
    _CACHE["last_res"] = res
    out = np.empty((B, N, D), np.float32)
    for b in range(B):
        out[b] = (res.results[2 * b]["o"].astype(np.float32)
                  + res.results[2 * b + 1]["o"].astype(np.float32))
    return out


# revision 57
# speedup vs baseline: 1.2614x; 1.0992x over previous
"""Causal multi-head attention (B=4, N=2048, D=768, H=12) on 8 TRN2 cores.

Sharding: batch (4) x head-split (2). Core c = (b, hg) handles batch b
and heads 6*hg .. 6*hg+5 over the FULL sequence: QKV projections take
only this half's weight columns, attention runs 6 heads, and the out
projection uses only this half's weight rows, producing a partial
[N, D] that the host sums across the pair of cores sharing a batch.

All matmul operands are bf16 (fp32 PSUM accumulation). Per core:
  qT/kT = Wqk^T x^T  ([2*64, N] head-pair tiles)    v = x Wv (+ones col)
  per (head, query-half, key-tile jt), software-pipelined so the PE
  never waits on the Act engine (scores for jt+1 issue before AV jt):
    s = kT_jt^T qT;  et = exp(s/8) bf16 (* 0/1 causal mask on diag
    tiles, DVE);  oT[65, :] += v_aug_jt^T et  (row 64 = denominators)
  aT = oT[0:64] (pair-packed via DMA repartition for odd heads)
  rec broadcast via PE selector-matmul into PSUM (deferred into the
  next head's loop);  aTb = aT * rec  bf16;   o_partial = aTb^T Wo
"""

import numpy as np

B, N, D, H = 4, 2048, 768, 12
DH = D // H          # 64
HH = H // 2          # 6 local heads per core
NPAIR = HH // 2      # 3 head pairs
KC = D // 128        # 6 contraction chunks
VW = HH * (DH + 1)   # 390 (v_aug row width per seq tile)
NEG = -30000.0

_CACHE = {}


def _build_nc(unused=None):
    import concourse.bacc as bacc
    import concourse.bass as bass
    import concourse.mybir as mybir
    import concourse.tile as tile
    from contextlib import ExitStack

    dt = mybir.dt
    f32 = dt.float32
    bf16 = dt.bfloat16
    fp8 = dt.float8e4
    Exp = mybir.ActivationFunctionType.Exp

    nc = bacc.Bacc(None)
    xt = nc.declare_dram_parameter("xt", [D, N], bf16, isOutput=False)
    wqk = nc.declare_dram_parameter("wqk", [D, 2 * HH * DH], bf16,
                                    isOutput=False)
    wv = nc.declare_dram_parameter("wv", [D, HH * DH], bf16, isOutput=False)
    wo = nc.declare_dram_parameter("wo", [HH * DH, D], bf16, isOutput=False)
    cst = nc.declare_dram_parameter("cst", [2, 128, 128], bf16, isOutput=False)
    o = nc.declare_dram_parameter("o", [N, D], bf16, isOutput=True)

    with tile.TileContext(nc) as tc:
        with ExitStack() as es:
            persist = es.enter_context(tc.tile_pool(name="persist", bufs=1))
            qT = [persist.tile([128, N], bf16, tag=f"qT{m}", name=f"qT{m}")
                  for m in range(NPAIR)]
            kT = [persist.tile([128, N], bf16, tag=f"kT{m}", name=f"kT{m}")
                  for m in range(NPAIR)]
            vp_all = persist.tile([128, 16 * VW], bf16, tag="vp", name="vp")
            vp = [vp_all[:, j * VW:(j + 1) * VW] for j in range(16)]
            aT = [persist.tile([128, N], f32, tag=f"aT{m}", name=f"aT{m}")
                  for m in range(NPAIR)]
            aTb = [persist.tile([128, N], bf16, tag=f"aTb{m}", name=f"aTb{m}")
                   for m in range(NPAIR)]
            # den/rec rows live at partitions 0 (even head) and 64 (odd
            # head) so the PE rec-broadcast lhsT/rhs base partitions align.
            den = [persist.tile([65, N], f32, tag=f"den{m}", name=f"den{m}")
                   for m in range(NPAIR)]
            rec = [persist.tile([65, N], f32, tag=f"rec{m}", name=f"rec{m}")
                   for m in range(NPAIR)]
            recb = persist.tile([128, N], bf16, tag="recb", name="recb")
            for m in range(NPAIR):
                nc.vector.memset(den[m][:], 1.0)
            nc.vector.memset(recb[:], 1.0)
            msk = persist.tile([128, 256], bf16, tag="msk", name="msk")
            tri01 = msk[:, 0:128]
            sel = msk[:, 128:256]
            wos = persist.tile([128, NPAIR * D], bf16, tag="wos", name="wos")

            # ---------------- projections ----------------
            with tc.tile_pool(name="xw", bufs=1) as xwp, \
                 tc.tile_pool(name="on", bufs=1) as onp, \
                 tc.tile_pool(name="pp", bufs=1, space="PSUM") as pp, \
                 tc.tile_pool(name="ppv", bufs=2, space="PSUM") as ppv:
                xts = xwp.tile([128, KC * N], bf16, tag="xts", name="xts")
                wqks = xwp.tile([128, KC * 768], bf16, tag="wqks",
                                name="wqks")
                wvs = xwp.tile([128, KC * 384], bf16, tag="wvs", name="wvs")
                for k in range(KC):
                    nc.scalar.dma_start(out=wqks[:, k * 768:(k + 1) * 768],
                                        in_=wqk[k * 128:(k + 1) * 128, :])
                    nc.sync.dma_start(
                        out=xts[:, k * N:k * N + 512],
                        in_=xt[k * 128:(k + 1) * 128, 0:512])

                # q/k projection, k-major: 6 output tiles accumulate in
                # PSUM while the k-chunk DMAs stream in behind them
                for qtr in range(4):
                    c0 = qtr * 512
                    if qtr > 0:
                        for k in range(KC):
                            nc.sync.dma_start(
                                out=xts[:, k * N + c0:k * N + c0 + 512],
                                in_=xt[k * 128:(k + 1) * 128, c0:c0 + 512])
                    pss = [pp.tile([128, 512], f32, tag=f"psq{m}",
                                   name=f"psq{m}") for m in range(2 * NPAIR)]
                    for k in range(KC):
                        for m in range(2 * NPAIR):
                            nc.tensor.matmul(
                                out=pss[m][:],
                                lhsT=wqks[:, k * 768 + m * 128:
                                          k * 768 + (m + 1) * 128],
                                rhs=xts[:, k * N + c0:k * N + c0 + 512],
                                start=(k == 0), stop=(k == KC - 1))
                    for m in range(2 * NPAIR):
                        dst = qT[m] if m < NPAIR else kT[m - NPAIR]
                        nc.vector.tensor_copy(dst[:, c0:c0 + 512],
                                              pss[m][:])
                for k in range(KC):
                    nc.gpsimd.dma_start(out=wvs[:, k * 384:(k + 1) * 384],
                                        in_=wv[k * 128:(k + 1) * 128, :])
                for i2 in range(2):
                    nc.sync.dma_start(out=msk[:, i2 * 128:(i2 + 1) * 128],
                                      in_=cst[i2])
                for f in range(NPAIR):
                    nc.gpsimd.dma_start(out=wos[:, f * D:(f + 1) * D],
                                        in_=wo[f * 128:(f + 1) * 128, :])

                # v projection: out [seq 128, 6*64] per seq tile
                for st in range(16):
                    ps = ppv.tile([128, 384], f32, tag="psv", name="psv")
                    for k in range(KC):
                        nc.tensor.matmul(
                            out=ps[:],
                            lhsT=xts[:, k * N + st * 128:
                                     k * N + (st + 1) * 128],
                            rhs=wvs[:, k * 384:(k + 1) * 384],
                            start=(k == 0), stop=(k == KC - 1))
                    vv = vp[st].rearrange("p (h c) -> p h c", c=DH + 1)
                    nc.vector.tensor_copy(
                        vv[:, :, 0:DH],
                        ps[:].rearrange("p (h c) -> p h c", c=DH))
                ones96 = onp.tile([128, 16 * HH], bf16, tag="on",
                                  name="ones96")
                nc.vector.memset(ones96[:], 1.0)
                vview = vp_all.rearrange("p (j c) -> p j c", c=DH + 1)
                nc.vector.tensor_copy(
                    vview[:, :, DH:DH + 1],
                    ones96[:].rearrange("p (a b) -> p a b", b=1))

            # ---------------- attention ----------------
            import concourse.bass as bass  # noqa: F401

            def r32(ap):
                return ap.bitcast(dt.float32r)

            with tc.tile_pool(name="et", bufs=5) as etp, \
                 tc.tile_pool(name="dtm", bufs=2) as dtp, \
                 tc.tile_pool(name="omp", bufs=2) as omp, \
                 tc.tile_pool(name="ps4", bufs=2, space="PSUM") as ps4, \
                 tc.tile_pool(name="po4", bufs=2, space="PSUM") as po4:
                # normalize PE-work (rec broadcast + multiply) is deferred
                # into a later jt loop so the PE never waits on the
                # DVE/DMA den->rec chain
                pnorm = []

                def flush_norm(pool):
                    while pnorm:
                        m_, q0_ = pnorm.pop(0)
                        rbps = pool.tile([128, 1024], f32, tag="ps",
                                         name="ps")
                        for (a, b) in ((0, 512), (512, 1024)):
                            nc.tensor.matmul(
                                out=rbps[:, a:b], lhsT=sel,
                                rhs=recb[:, q0_ + a:q0_ + b],
                                start=True, stop=True,
                                skip_group_check=True)
                        nc.vector.tensor_mul(
                            aTb[m_][:, q0_:q0_ + 1024],
                            aT[m_][:, q0_:q0_ + 1024], rbps[:])

                for m in range(NPAIR):
                    otmp = omp.tile([64, N], f32, tag="otmp", name="otmp")
                    for hh in range(2):
                        h = 2 * m + hh
                        hs = slice(hh * 64, hh * 64 + 64)
                        for qc in range(2):
                            q0 = qc * 1024
                            oT = po4.tile([65, 1024], f32, tag="oT",
                                          name="oT")
                            njt = 8 * qc + 8

                            def do_av(jt, et, regs):
                                for (a, b) in regs:
                                    nc.tensor.matmul(
                                        out=oT[:, a:b],
                                        lhsT=vp[jt][:, h * (DH + 1):
                                                    (h + 1) * (DH + 1)],
                                        rhs=et[:, a:b],
                                        start=(jt == 0),
                                        stop=(jt == (8 * qc + 3 if b <= 512
                                                     else njt - 1)),
                                        skip_group_check=True)

                            # jt-pair bursting: S,S then the previous
                            # pair's AV,AV — fewer stationary switches and
                            # semaphore-gated PE instructions per step
                            pend = []
                            for jt in range(njt):
                                ql0 = max(0, 128 * jt - q0)
                                diag = jt >= 8 * qc
                                regs = []
                                if ql0 < 512:
                                    regs.append((ql0, 512))
                                regs.append((max(ql0, 512), 1024))
                                if jt == 6:
                                    flush_norm(ps4)
                                ps = ps4.tile([128, 1024], f32, tag="ps",
                                              name="ps")
                                for (a, b) in regs:
                                    nc.tensor.matmul(
                                        out=ps[:, a:b],
                                        lhsT=kT[m][hs, jt * 128:
                                                   (jt + 1) * 128],
                                        rhs=qT[m][hs, q0 + a:q0 + b],
                                        start=True, stop=True,
                                        skip_group_check=True)
                                et = etp.tile([128, 1024], bf16, tag="et",
                                              name="et")
                                nc.scalar.activation(
                                    out=et[:, ql0:1024], in_=ps[:, ql0:1024],
                                    func=Exp, scale=0.125)
                                if diag:
                                    nc.vector.tensor_mul(
                                        et[:, ql0:ql0 + 128],
                                        et[:, ql0:ql0 + 128], tri01)
                                pend.append((jt, et, regs))
                                if jt % 2 == 1 and len(pend) == 4:
                                    do_av(*pend.pop(0))
                                    do_av(*pend.pop(0))
                            for p_ in pend:
                                do_av(*p_)
                            # drain oT: rows 0..63 -> aT / otmp, row 64 -> den
                            if hh == 0:
                                nc.vector.tensor_copy(
                                    aT[m][0:64, q0:q0 + 1024], oT[0:64, :])
                            else:
                                nc.vector.tensor_copy(
                                    otmp[0:64, q0:q0 + 1024], oT[0:64, :])
                            dtm = dtp.tile([65, 1024], f32, tag="dtm",
                                           name="dtm")
                            nc.vector.tensor_copy(dtm[64:65, :], oT[64:65, :])
                            dr = hh * 64  # den row: partition 0 / 64
                            nc.sync.dma_start(
                                out=den[m][dr:dr + 1, q0:q0 + 1024],
                                in_=dtm[64:65, :])
                            if hh == 1:
                                # pair half complete: repartition odd head,
                                # then normalize this query-half
                                nc.sync.dma_start(
                                    out=aT[m][64:128, q0:q0 + 1024],
                                    in_=otmp[:, q0:q0 + 1024])
                                nc.vector.reciprocal_approx_fast(
                                    out=rec[m][:, q0:q0 + 1024],
                                    in_=den[m][:, q0:q0 + 1024])
                                nc.vector.tensor_copy(
                                    recb[0:65, q0:q0 + 1024],
                                    rec[m][:, q0:q0 + 1024])
                                pnorm.append((m, q0))

            # ---------------- output projection ----------------
            with tc.tile_pool(name="osb", bufs=3) as osb, \
                 tc.tile_pool(name="rbf", bufs=1, space="PSUM") as rbf, \
                 tc.tile_pool(name="pp5", bufs=3, space="PSUM") as pp5:
                for st in range(16):
                    if st == 1:
                        # last pair's deferred normalize: its PE ops land
                        # here so st=0's matmuls cover the DVE chain; only
                        # st>=8 reads the columns this mul produces
                        flush_norm(rbf)
                    ps = pp5.tile([128, D], f32, tag="ps5", name="ps5")
                    for f in range(NPAIR):
                        for (a, b) in ((0, 512), (512, 768)):
                            nc.tensor.matmul(
                                out=ps[:, a:b],
                                lhsT=aTb[f][:, st * 128:(st + 1) * 128],
                                rhs=wos[:, f * D + a:f * D + b],
                                start=(f == 0), stop=(f == NPAIR - 1))
                    ot = osb.tile([128, D], bf16, tag="ot", name="ot")
                    nc.vector.tensor_copy(ot[:], ps[:])
                    nc.sync.dma_start(out=o[st * 128:(st + 1) * 128, :],
                                      in_=ot[:])

    nc.finalize()
    return nc


def _mask_tiles():
    import ml_dtypes
    # tri01[jp, q] = 1 iff key jp <= query q (within the diagonal tile)
    tri01 = np.triu(np.ones((128, 128), np.float32))
    # sel broadcasts rec rows (0 -> out 0..63, 64 -> out 64..127)
    sel = np.zeros((128, 128), np.float32)
    sel[0, 0:64] = 1.0
    sel[64, 64:128] = 1.0
    return np.stack([tri01, sel]).astype(ml_dtypes.bfloat16)


def _host_reference(x, mask, w_qkv, w_out):
    qkv = x.astype(np.float64) @ w_qkv.astype(np.float64)
    q, k, v = np.split(qkv, 3, axis=-1)

    def heads(t):
        return t.reshape(B, N, H, DH).transpose(0, 2, 1, 3)
    q, k, v = heads(q), heads(k), heads(v)
    s = np.einsum('bhqd,bhkd->bhqk', q, k) / np.sqrt(DH)
    s = np.where(np.asarray(mask).reshape(1, 1, N, N) == 0, -np.inf, s)
    s = s - s.max(-1, keepdims=True)
    e = np.exp(s)
    p = e / e.sum(-1, keepdims=True)
    out = np.einsum('bhqk,bhkd->bhqd', p, v)
    out = out.transpose(0, 2, 1, 3).reshape(B, N, D)
    return (out @ w_out.astype(np.float64)).astype(np.float32)


def kernel(x, mask, w_qkv, w_out):
    import ml_dtypes
    bf = ml_dtypes.bfloat16
    x = np.asarray(x)
    w_qkv = np.asarray(w_qkv)
    w_out = np.asarray(w_out)

    causal = np.array_equal(
        np.asarray(mask).reshape(N, N) != 0, np.tril(np.ones((N, N), bool)))
    if not causal:
        return _host_reference(x, mask, w_qkv, w_out)

    from concourse.bass_utils import run_bass_kernel_spmd
    if "nc" not in _CACHE:
        _CACHE["nc"] = _build_nc()
    nc = _CACHE["nc"]

    cstn = _mask_tiles()
    W = HH * DH  # 384
    wqk_h, wv_h, wo_h = [], [], []
    for hg in range(2):
        wqk_h.append(np.ascontiguousarray(np.concatenate(
            [w_qkv[:, hg * W:(hg + 1) * W],
             w_qkv[:, D + hg * W:D + (hg + 1) * W]], axis=1)).astype(bf))
        wv_h.append(np.ascontiguousarray(
            w_qkv[:, 2 * D + hg * W:2 * D + (hg + 1) * W]).astype(bf))
        wo_h.append(np.ascontiguousarray(
            w_out[hg * W:(hg + 1) * W, :]).astype(bf))
    xts = [np.ascontiguousarray(x[b].T).astype(bf) for b in range(B)]

    in_maps = []
    for c in range(8):
        b, hg = c // 2, c % 2
        in_maps.append({
            "xt": xts[b],
            "wqk": wqk_h[hg], "wv": wv_h[hg], "wo": wo_h[hg],
            "cst": cstn,
        })
    res = run_bass_kernel_spmd(nc, in_maps, core_ids=list(range(8)),
                               **_CACHE.get("run_kwargs", {}))
    _CACHE["last_res"] = res
    out = np.empty((B, N, D), np.float32)
    for b in range(B):
        out[b] = (res.results[2 * b]["o"].astype(np.float32)
                  + res.results[2 * b + 1]["o"].astype(np.float32))
    return out
